# revision 1
# baseline (speedup 1.0000x reference)
"""Multi-head self-attention (B=2, T=2048, D=1024, 16 heads) on 8 TRN2 cores.

Sharding: core c = (b, g) with b = c // 4 (batch), g = c % 4 (head group of 4).
Each core computes q/k/v projections for its 4 heads, causal softmax
attention, and a partial output projection (its 256 columns of the
concat-head dim against Wo). Host sums the 4 partials per batch and adds bo.

All matmuls run in float32r (TF32-like, full PE rate at free dim >= 256).
DRAM inputs are declared float32r so they DMA straight into matmul operands.

Per-core pipeline:
  phase A: qT/kT [256,2048] (transposed projections, head pairs stacked on
           partitions) and v [2048,260] natural (per head 64 value cols + a
           ones col that makes the AV matmul emit softmax denominators).
  phase B: per (head-pair hp, query block J of 512): scoresT chunks
           [tk=128, tq<=512] = kT.T @ qT row-packed via tile_position
           (0,0)/(64,0); ACT exp(0.125 x) PSUM->SBUF (off-diagonal chunks
           paired into [128,1024] PSUM tiles, one ACT instr); diagonal chunks
           column-restricted to the causal region and masked with a single
           [128,128] lower-tri multiply; AV accumulates [v|1].T @ exp into
           attT [65, 512]; row 64 holds denominators; normalize via
           reciprocal_approx_fast + gpsimd partition broadcast + DVE mult.
  phase C: O [2048,1024] = attT.T @ WoS, DVE PSUM->SBUF copies, DMA out.
"""

import ml_dtypes
import numpy as np

import concourse.bass as bass
import concourse.tile as tile
from concourse import bacc, mybir
from concourse import bass_utils
from contextlib import ExitStack

F32 = mybir.dt.float32
F32R = mybir.dt.float32r
BF16 = mybir.dt.bfloat16
ATT = BF16  # dtype for attention-phase matmul operands
AF = mybir.ActivationFunctionType
OP = mybir.AluOpType

B, T, D = 2, 2048, 1024
NH, DH = 16, 64
HPC = 4            # heads per core
GD = HPC * DH      # 256, group dim
GV = HPC * (DH + 1)  # 260, v tile width
NKD = D // 128     # 8 K-chunks for projections
NT = T // 128      # 16 token chunks
NJ = T // 512      # 4 query blocks

_NC_CACHE = {}


def build():
    if "nc" in _NC_CACHE:
        return _NC_CACHE["nc"]
    nc = bacc.Bacc("TRN2", target_bir_lowering=False, debug=False, num_devices=8)

    HT = nc.dram_tensor("HT", [D, T], BF16, kind="ExternalInput").ap()
    WqT = nc.dram_tensor("WqT", [D, GD], BF16, kind="ExternalInput").ap()
    WkT = nc.dram_tensor("WkT", [D, GD], BF16, kind="ExternalInput").ap()
    WvS = nc.dram_tensor("WvS", [D, GV], BF16, kind="ExternalInput").ap()
    WoS = nc.dram_tensor("WoS", [GD, D], F32R, kind="ExternalInput").ap()
    bq = nc.dram_tensor("bq", [1, GD], BF16, kind="ExternalInput").ap()
    bk = nc.dram_tensor("bk", [1, GD], BF16, kind="ExternalInput").ap()
    bvS = nc.dram_tensor("bvS", [1, GV], BF16, kind="ExternalInput").ap()
    kpm = nc.dram_tensor("kpm", [128, NT], F32, kind="ExternalInput").ap()
    O = nc.dram_tensor("O", [T, D], F32, kind="ExternalOutput").ap()
    sums_d = nc.dram_tensor("sums_d", [16, 512], F32, kind="Internal").ap()
    recip_d = nc.dram_tensor("recip_d", [16, 512], F32, kind="Internal").ap()

    ENGS = [nc.sync, nc.scalar, nc.gpsimd]

    with tile.TileContext(nc) as tc, ExitStack() as octx:
        cpool = octx.enter_context(tc.tile_pool(name="const", bufs=1))
        keep = octx.enter_context(tc.tile_pool(name="keep", bufs=1))

        # ---- constants ----
        ones_f = cpool.tile([1, 512], F32, name="ones_f", tag="ones_f")
        nc.vector.memset(ones_f[:], 1.0)
        ones_r = cpool.tile([1, 512], BF16, name="ones_r", tag="ones_r")
        nc.vector.tensor_copy(ones_r[:], ones_f[:])

        bq_r = cpool.tile([1, GD], BF16, name="bq_r", tag="bq_r")
        bk_r = cpool.tile([1, GD], BF16, name="bk_r", tag="bk_r")
        bv_r = cpool.tile([1, GV], BF16, name="bv_r", tag="bv_r")
        nc.sync.dma_start(bq_r[:], bq[:])
        nc.sync.dma_start(bk_r[:], bk[:])
        nc.sync.dma_start(bv_r[:], bvS[:])

        kpm_sb = cpool.tile([128, NT], F32, name="kpm_sb", tag="kpm_sb")
        nc.sync.dma_start(kpm_sb[:], kpm[:])

        zc = cpool.tile([128, 1], F32, name="zc", tag="zc")
        nc.vector.memset(zc[:], 0.0)



        # lower-tri mask [128,128]: keep where f >= p
        tri = cpool.tile([128, 128], ATT, name="tri", tag="tri")
        nc.gpsimd.memset(tri[:], 1.0)
        nc.gpsimd.affine_select(
            out=tri[:], in_=tri[:], compare_op=OP.is_ge, fill=0.0,
            base=0, pattern=[[1, 128]], channel_multiplier=-1,
        )

        # ---- long-lived activations ----
        qT = [keep.tile([128, T], ATT, name=f"qT{m}", tag=f"qT{m}") for m in range(2)]
        kT = [keep.tile([128, T], ATT, name=f"kT{m}", tag=f"kT{m}") for m in range(2)]
        vt = [keep.tile([128, 512], ATT, name=f"vt{t}", tag=f"vt{t}") for t in range(NT)]
        attT = [keep.tile([128, T], F32R, name=f"attT{m}", tag=f"attT{m}") for m in range(2)]
        wo_r = [keep.tile([128, D], F32R, name=f"wo{i}", tag=f"wo{i}") for i in range(2)]
        for i in range(2):
            nc.scalar.dma_start(wo_r[i][:], WoS[i * 128:(i + 1) * 128, :])

        # ================= phase A: projections =================
        with ExitStack() as actx:
            apool = actx.enter_context(tc.tile_pool(name="phA", bufs=1))
            apsum = actx.enter_context(tc.tile_pool(name="phA_ps", bufs=1, space="PSUM"))

            ht_r = [apool.tile([128, T], BF16, name=f"ht{k}", tag=f"ht{k}") for k in range(NKD)]
            wq_r = apool.tile([128, NKD * GD], BF16, name="wq_r", tag="wq_r")
            wk_r = apool.tile([128, NKD * GD], BF16, name="wk_r", tag="wk_r")
            wv_r = apool.tile([128, NKD * GV], BF16, name="wv_r", tag="wv_r")
            # interleave weight + HT chunk loads across engine DMA queues
            for k in range(NKD):
                e = ENGS[k % len(ENGS)]
                e.dma_start(wq_r[:, k * GD:(k + 1) * GD], WqT[k * 128:(k + 1) * 128, :])
                e.dma_start(wk_r[:, k * GD:(k + 1) * GD], WkT[k * 128:(k + 1) * 128, :])
                e.dma_start(wv_r[:, k * GV:(k + 1) * GV], WvS[k * 128:(k + 1) * 128, :])
                ENGS[(k + 2) % len(ENGS)].dma_start(ht_r[k][:], HT[k * 128:(k + 1) * 128, :])

            # qT / kT: out[dq(128), t(512)] = sum_k WT_k_m.T @ HT_k + bias
            for w_r, dest, brow in ((wq_r, qT, bq_r), (wk_r, kT, bk_r)):
                for m in range(2):
                    ps = [
                        apsum.tile([128, 512], F32, name=f"pp{n}", tag="proj", bufs=6)
                        for n in range(4)
                    ]
                    for k in range(NKD):
                        for n in range(4):
                            nc.tensor.matmul(
                                ps[n][:],
                                w_r[:, k * GD + m * 128: k * GD + m * 128 + 128],
                                ht_r[k][:, n * 512:(n + 1) * 512],
                                start=(k == 0), stop=False,
                            )
                    for n in range(4):
                        nc.tensor.matmul(
                            ps[n][:],
                            brow[:, m * 128:(m + 1) * 128],
                            ones_r[:],
                            start=False, stop=True,
                        )
                        nc.scalar.copy(dest[m][:, n * 512:(n + 1) * 512], ps[n][:])

            # v natural: out[t(128), 260] = sum_k HT_k_t.T @ WvS_k + ones.T @ bvS
            for t in range(NT):
                vp = apsum.tile([128, GV], F32, name="vp", tag="vps", bufs=2)
                for k in range(NKD):
                    nc.tensor.matmul(
                        vp[:],
                        ht_r[k][:, t * 128:(t + 1) * 128],
                        wv_r[:, k * GV:(k + 1) * GV],
                        start=(k == 0), stop=False,
                    )
                nc.tensor.matmul(
                    vp[:], ones_r[:, 0:128], bv_r[:], start=False, stop=True
                )
                nc.vector.memset(vt[t][:], 0.0)
                nc.scalar.copy(
                    vt[t][:].rearrange("p (h c) -> p h c", c=128)[:, :, 0:65],
                    vp[:].rearrange("p (h c) -> p h c", c=65),
                )
                nc.vector.tensor_scalar_mul(vt[t][:], vt[t][:], kpm_sb[:, t:t + 1])

        # ================= phase B: attention =================
        with ExitStack() as bctx:
            bpool = bctx.enter_context(tc.tile_pool(name="phB", bufs=1))
            bpsum = bctx.enter_context(tc.tile_pool(name="phB_ps", bufs=1, space="PSUM"))

            def normalize(hp, J, at, bi):
                # softmax denominators: bounce the PSUM sums row through DRAM
                # into a [128, 4] partition-packed tile so the exact DVE
                # reciprocal runs on 4 elements/partition instead of 512;
                # ACT stays exp-only (no LUT-set reloads). The unnormalized
                # attT rows are copied out first so the PSUM bank frees fast;
                # the normalize multiply runs in place afterwards.
                def act_reciprocal(out, in_):
                    # ACT-LUT reciprocal: ~1e-5 relative error for softmax
                    # denominators (all >= 1.0); keeps the slow exact
                    # reciprocal off the vector engine.
                    eng = nc.scalar
                    ins = [eng.lower_ap(in_)]
                    for arg in (0.0, 1.0, 0.0):  # bias, scale, alpha
                        ins.append(mybir.ImmediateValue(dtype=F32, value=arg))
                    eng.add_instruction(mybir.InstActivation(
                        name=nc.get_next_instruction_name(),
                        func=AF.Reciprocal,
                        ins=ins,
                        outs=[eng.lower_ap(out)],
                    ))

                aus = []
                for hh in range(2):
                    au = bpool.tile([64, 512], F32, name="au", tag="au", bufs=8)
                    nc.vector.tensor_copy(au[:], at[hh][0:64, :])
                    srow = bpool.tile([1, 512], F32, name="srow", tag="srow", bufs=8)
                    nc.vector.tensor_copy(srow[:], at[hh][64:65, :])
                    aus.append((au, srow))
                for hh in range(2):
                    au, srow = aus[hh]
                    rc = bpool.tile([1, 512], F32, name="rc", tag="rc", bufs=8)
                    act_reciprocal(rc[:], srow[:])
                    rb = bpool.tile([64, 512], F32, name="rb", tag="rb", bufs=8)
                    nc.gpsimd.partition_broadcast(rb[:], rc[:])
                    nc.vector.tensor_tensor(
                        attT[hp][hh * 64:(hh + 1) * 64, J * 512:(J + 1) * 512],
                        au[:],
                        rb[:],
                        op=OP.mult,
                    )

            pending_norm = None
            for bi, (J, hp) in enumerate([(3, 0), (0, 0), (3, 1), (0, 1), (2, 0), (1, 0), (2, 1), (1, 1)]):
                if True:
                    n_kc = 4 * J + 4
                    at = [
                        bpsum.tile([128, 512], F32, name=f"at{hh}", tag="av", bufs=4)
                        for hh in range(2)
                    ]

                    # per key-chunk kc: one [128, 2w] PSUM tile holds both
                    # heads' scoresT side by side; the two scores matmuls are
                    # adjacent with distinct row groups (concurrent on PE);
                    # one ACT exp covers both. Diagonal chunks (kc >= 4J) are
                    # column-restricted to the causal region (width w) and
                    # tri-masked on their first 128 columns.
                    # Order: diagonal i=0 first (full width, opens the PSUM
                    # accumulation), then off-diagonals, then narrow diagonals.
                    kcs = [4 * J] + list(range(4 * J)) + [4 * J + i for i in range(1, 4)]

                    def issue_sc_exp(kc):
                        off = max(0, 128 * (kc - 4 * J))
                        w = 512 - off
                        # both heads in one 2-bank tile, each half bank-aligned
                        sc = bpsum.tile([128, 1024], F32, name="sc", tag="sc", bufs=2)
                        for hh in range(2):
                            nc.tensor.matmul(
                                sc[:, hh * 512:hh * 512 + w],
                                kT[hp][hh * 64:(hh + 1) * 64, kc * 128:(kc + 1) * 128],
                                qT[hp][hh * 64:(hh + 1) * 64, J * 512 + off:(J + 1) * 512],
                                start=True, stop=True,
                                tile_position=(hh * 64, 0),
                            )
                        ex = bpool.tile([128, 1024], ATT, name="ex", tag="ex", bufs=8)
                        nc.scalar.activation(
                            ex[:].rearrange("p (h c) -> p h c", c=512)[:, :, 0:w],
                            sc[:].rearrange("p (h c) -> p h c", c=512)[:, :, 0:w],
                            AF.Exp, scale=0.125,
                        )
                        if off or kc == 4 * J:
                            for hh in range(2):
                                nc.vector.tensor_tensor(
                                    ex[:, hh * 512:hh * 512 + 128],
                                    ex[:, hh * 512:hh * 512 + 128],
                                    tri[:],
                                    op=OP.mult,
                                )
                        return ex

                    def issue_av(kc, ex, first, last):
                        off = max(0, 128 * (kc - 4 * J))
                        w = 512 - off
                        for hh in range(2):
                            h = 2 * hp + hh
                            nc.tensor.matmul(
                                at[hh][:, off:512],
                                vt[kc][:, h * 128:(h + 1) * 128],
                                ex[:, hh * 512:hh * 512 + w],
                                start=first, stop=last,
                            )

                    # software pipeline: scores/exp one chunk ahead of AV; the
                    # previous block's normalize chain is issued after the
                    # first chunk so its slow reciprocal overlaps the
                    # mask-free off-diagonal stretch
                    prev = None
                    for ti, kc in enumerate(kcs):
                        ex = issue_sc_exp(kc)
                        if ti == 1 and pending_norm is not None:
                            normalize(*pending_norm)
                            pending_norm = None
                        if prev is not None:
                            issue_av(prev[0], prev[1], first=(prev[2] == 0), last=False)
                        prev = (kc, ex, ti)
                    issue_av(prev[0], prev[1], first=(prev[2] == 0), last=True)

                    if pending_norm is not None:
                        normalize(*pending_norm)
                    pending_norm = (hp, J, at, bi)
            normalize(*pending_norm)


        # ================= phase C: output projection =================
        with ExitStack() as cctx:
            opool = cctx.enter_context(tc.tile_pool(name="phC", bufs=1))
            opsum = cctx.enter_context(tc.tile_pool(name="phC_ps", bufs=1, space="PSUM"))
            for t in range(NT):
                ot = opool.tile([128, D], F32, name="ot", tag="ot", bufs=3)
                for n in range(2):
                    op = opsum.tile([128, 512], F32, name="op", tag="op", bufs=4)
                    for hp in range(2):
                        nc.tensor.matmul(
                            op[:],
                            attT[hp][:, t * 128:(t + 1) * 128],
                            wo_r[hp][:, n * 512:(n + 1) * 512],
                            start=(hp == 0), stop=(hp == 1),
                        )
                    if n == 0:
                        nc.vector.tensor_copy(ot[:, 0:512], op[:])
                    else:
                        nc.scalar.copy(ot[:, 512:1024], op[:])
                nc.sync.dma_start(O[t * 128:(t + 1) * 128, :], ot[:])

    nc.compile()
    _NC_CACHE["nc"] = nc
    return nc


def _prep_core_inputs(H, key_padding_mask, Wq, bq, Wk, bk, Wv, bv, Wo, bo):
    keep = 1.0 - np.asarray(key_padding_mask, dtype=np.float32)  # [B, T]
    in_maps = []
    for c in range(8):
        b, g = divmod(c, 4)
        sl = slice(g * GD, (g + 1) * GD)
        WvT = Wv[sl].T  # [D, GD]
        WvS = np.zeros((D, GV), dtype=np.float32)
        bvS = np.zeros((1, GV), dtype=np.float32)
        for h in range(HPC):
            WvS[:, h * 65:h * 65 + 64] = WvT[:, h * 64:(h + 1) * 64]
            bvS[0, h * 65:h * 65 + 64] = bv[sl][h * 64:(h + 1) * 64]
            bvS[0, h * 65 + 64] = 1.0
        bf = ml_dtypes.bfloat16
        in_maps.append({
            "HT": np.ascontiguousarray(H[b].T).astype(bf),
            "WqT": np.ascontiguousarray(Wq[sl].T).astype(bf),
            "WkT": np.ascontiguousarray(Wk[sl].T).astype(bf),
            "WvS": WvS.astype(bf),
            "WoS": np.ascontiguousarray(Wo[:, sl].T),
            "bq": np.ascontiguousarray(bq[sl][None, :]).astype(bf),
            "bk": np.ascontiguousarray(bk[sl][None, :]).astype(bf),
            "bvS": bvS.astype(bf),
            "kpm": np.ascontiguousarray(keep[b].reshape(NT, 128).T),
        })
    return in_maps


def kernel(H, key_padding_mask, Wq, bq, Wk, bk, Wv, bv, Wo, bo, _run_kwargs=None):
    H = np.asarray(H, dtype=np.float32)
    Wq = np.asarray(Wq, dtype=np.float32)
    Wk = np.asarray(Wk, dtype=np.float32)
    Wv = np.asarray(Wv, dtype=np.float32)
    Wo = np.asarray(Wo, dtype=np.float32)
    bq = np.asarray(bq, dtype=np.float32)
    bk = np.asarray(bk, dtype=np.float32)
    bv = np.asarray(bv, dtype=np.float32)
    bo = np.asarray(bo, dtype=np.float32)

    nc = build()
    in_maps = _prep_core_inputs(H, key_padding_mask, Wq, bq, Wk, bk, Wv, bv, Wo, bo)
    res = bass_utils.run_bass_kernel_spmd(
        nc, in_maps, core_ids=list(range(8)), **(_run_kwargs or {})
    )
    out = np.zeros((B, T, D), dtype=np.float32)
    for c in range(8):
        out[c // 4] += res.results[c]["O"]
    out += bo
    if _run_kwargs:
        kernel.last_result = res
    return out



# revision 5
# speedup vs baseline: 1.1564x; 1.1564x over previous
"""Multi-head self-attention (B=2, T=2048, D=1024, 16 heads) on 8 TRN2 cores.

Sharding: core c = (b, g) with b = c // 4 (batch), g = c % 4 (head group of 4).
Each core computes q/k/v projections for its 4 heads, causal softmax
attention, and a partial output projection (its 256 columns of the
concat-head dim against Wo). Host sums the 4 partials per batch and adds bo.

Single interleaved schedule built around the scalar (ACT) engine's exp
throughput, which is the binding serial resource of the attention middle:

  - ACT runs ONLY Exp instructions (one LUT load total). All PSUM->SBUF
    moves ride the vector engine with bias-add / padding-mask folded into
    tensor_scalar ops; softmax normalization is DVE row-copy -> gpsimd
    partition_broadcast -> DVE divide (no ACT Reciprocal, no LUT switches).
  - The attention blocks (J = query block of 512, hp = head pair) are
    emitted in causal order; v-projection chunks 8..15, q/k projections for
    token halves 2..3, and output-projection chunks are interleaved into
    the exp-bound stream as PE filler so the tensor engine never idles long
    enough to lose its HAM clock (stays at 2.4 GHz).
  - PSUM is budgeted exactly: scores 2x[128,1024] (4 banks) + attn-out
    2x[128,512] (2 banks) + a shared aux tag (2 banks) used in turn by the
    qk projection accumulators, v accumulators, and O-projection tiles.
  - O is stored bf16 (halves output DMA); host upcasts, sums partials, +bo.

Per-block attention math is the baseline scheme: scoresT chunks
[tk=128, tq<=512] = kT.T @ qT with both heads of the pair row-packed via
tile_position (0,0)/(64,0); ACT exp(0.125 x) PSUM->SBUF bf16; diagonal
chunks column-restricted to the causal region and tri-masked on their
first 128 columns; AV accumulates [v|1].T @ exp into at [65, 512] whose
row 64 is the softmax denominator (the ones column of v, masked by the
key-padding keep mask, also makes masked keys vanish from the sum).
"""

import ml_dtypes
import numpy as np

import concourse.bass as bass
import concourse.tile as tile
from concourse import bacc, mybir
from concourse import bass_utils
from contextlib import ExitStack

F32 = mybir.dt.float32
BF16 = mybir.dt.bfloat16
AF = mybir.ActivationFunctionType
OP = mybir.AluOpType

B, T, D = 2, 2048, 1024
NH, DH = 16, 64
HPC = 4              # heads per core
GD = HPC * DH        # 256, group dim
GV = HPC * (DH + 1)  # 260, packed v width (64 v dims + ones col per head)
NKD = D // 128       # 8 K-chunks for projections
NT = T // 128        # 16 token chunks
NJ = T // 512        # 4 query blocks

_NC_CACHE = {}


def build():
    if "nc" in _NC_CACHE:
        return _NC_CACHE["nc"]
    nc = bacc.Bacc("TRN2", target_bir_lowering=False, debug=False, num_devices=8)

    HT = nc.dram_tensor("HT", [D, T], BF16, kind="ExternalInput").ap()
    WqP = nc.dram_tensor("WqP", [128, NKD * GD], BF16, kind="ExternalInput").ap()
    WkP = nc.dram_tensor("WkP", [128, NKD * GD], BF16, kind="ExternalInput").ap()
    WvP = nc.dram_tensor("WvP", [128, NKD * GV], BF16, kind="ExternalInput").ap()
    WoP = nc.dram_tensor("WoP", [128, 2 * D], BF16, kind="ExternalInput").ap()
    bqk = nc.dram_tensor("bqk", [128, 4], F32, kind="ExternalInput").ap()
    bvP = nc.dram_tensor("bvP", [1, GV], BF16, kind="ExternalInput").ap()
    kpm = nc.dram_tensor("kpm", [128, NT], F32, kind="ExternalInput").ap()
    O = nc.dram_tensor("O", [T, D], BF16, kind="ExternalOutput").ap()

    with tile.TileContext(nc) as tc, ExitStack() as octx:
        cpool = octx.enter_context(tc.tile_pool(name="const", bufs=1))
        keep = octx.enter_context(tc.tile_pool(name="keep", bufs=1))
        work = octx.enter_context(tc.tile_pool(name="work", bufs=1))
        psc = octx.enter_context(tc.tile_pool(name="psc", bufs=1, space="PSUM"))
        pav = octx.enter_context(tc.tile_pool(name="pav", bufs=1, space="PSUM"))
        paux = octx.enter_context(tc.tile_pool(name="paux", bufs=1, space="PSUM"))

        # ---- constants (small DMAs on the gpsimd queue) ----
        ones_r = cpool.tile([1, 128], BF16, name="ones_r", tag="ones_r")
        nc.vector.memset(ones_r[:], 1.0)
        bqk_sb = cpool.tile([128, 4], F32, name="bqk_sb", tag="bqk_sb")
        nc.gpsimd.dma_start(bqk_sb[:], bqk[:])
        bv_r = cpool.tile([1, GV], BF16, name="bv_r", tag="bv_r")
        nc.gpsimd.dma_start(bv_r[:], bvP[:])
        kpm_sb = cpool.tile([128, NT], F32, name="kpm_sb", tag="kpm_sb")
        nc.gpsimd.dma_start(kpm_sb[:], kpm[:])

        # lower-tri mask [128,128]: keep where free >= partition
        tri = cpool.tile([128, 128], BF16, name="tri", tag="tri")
        nc.gpsimd.memset(tri[:], 1.0)
        nc.gpsimd.affine_select(
            out=tri[:], in_=tri[:], compare_op=OP.is_ge, fill=0.0,
            base=0, pattern=[[1, 128]], channel_multiplier=-1,
        )

        # ---- long-lived tiles ----
        qT = [keep.tile([128, T], BF16, name=f"qT{m}", tag=f"qT{m}") for m in range(2)]
        kT = [keep.tile([128, T], BF16, name=f"kT{m}", tag=f"kT{m}") for m in range(2)]
        vt = [keep.tile([128, GV], BF16, name=f"vt{t}", tag=f"vt{t}") for t in range(NT)]
        attT = [keep.tile([128, T], BF16, name=f"attT{m}", tag=f"attT{m}") for m in range(2)]
        ht_r = [keep.tile([128, T], BF16, name=f"ht{k}", tag=f"ht{k}") for k in range(NKD)]
        wq_r = keep.tile([128, NKD * GD], BF16, name="wq_r", tag="wq_r")
        wk_r = keep.tile([128, NKD * GD], BF16, name="wk_r", tag="wk_r")
        wv_r = keep.tile([128, NKD * GV], BF16, name="wv_r", tag="wv_r")
        wo_r = keep.tile([128, 2 * D], BF16, name="wo_r", tag="wo_r")

        # ---- input DMA: sync gets the first-half needs, gpsimd the rest ----
        nc.sync.dma_start(wq_r[:], WqP[:])
        for k in range(NKD):
            nc.sync.dma_start(ht_r[k][:, 0:1024], HT[k * 128:(k + 1) * 128, 0:1024])
        nc.sync.dma_start(wk_r[:], WkP[:])
        nc.gpsimd.dma_start(wv_r[:], WvP[:])
        for k in range(NKD):
            nc.gpsimd.dma_start(ht_r[k][:, 1024:2048], HT[k * 128:(k + 1) * 128, 1024:2048])
        nc.gpsimd.dma_start(wo_r[:], WoP[:])

        # ---- emission helpers ----
        def qk_unit(dest, w_r, m, n, bcol):
            # dest[m][:, n*512:(n+1)*512] = sum_k W_k_m.T @ ht_k_n + bias[:, bcol]
            ps = paux.tile([128, 512], F32, name="ps", tag="aux", bufs=2)
            for k in range(NKD):
                nc.tensor.matmul(
                    ps[:],
                    w_r[:, k * GD + m * 128: k * GD + m * 128 + 128],
                    ht_r[k][:, n * 512:(n + 1) * 512],
                    start=(k == 0), stop=(k == NKD - 1),
                )
            nc.vector.tensor_scalar_add(
                dest[m][:, n * 512:(n + 1) * 512], ps[:], bqk_sb[:, bcol:bcol + 1]
            )

        def v_unit(t):
            # vt[t][128 keys, 260] = (sum_k ht_k_t.T @ WvP_k + ones.T @ bvP) * kpm[:, t]
            vp = paux.tile([128, 512], F32, name="vp", tag="aux", bufs=2)
            for k in range(NKD):
                nc.tensor.matmul(
                    vp[:, 0:GV],
                    ht_r[k][:, t * 128:(t + 1) * 128],
                    wv_r[:, k * GV:(k + 1) * GV],
                    start=(k == 0), stop=False,
                )
            nc.tensor.matmul(
                vp[:, 0:GV], ones_r[:], bv_r[:], start=False, stop=True
            )
            nc.vector.tensor_scalar_mul(vt[t][:], vp[:, 0:GV], kpm_sb[:, t:t + 1])

        def c_unit(t):
            # O[t*128:(t+1)*128, :] = sum_hp attT[hp][:, t-slice].T @ WoP_hp
            ot = work.tile([128, D], BF16, name="ot", tag="ot", bufs=3)
            for n in range(2):
                op = paux.tile([128, 512], F32, name="op", tag="aux", bufs=2)
                for hp in range(2):
                    nc.tensor.matmul(
                        op[:],
                        attT[hp][:, t * 128:(t + 1) * 128],
                        wo_r[:, hp * D + n * 512: hp * D + (n + 1) * 512],
                        start=(hp == 0), stop=(hp == 1),
                    )
                nc.vector.tensor_copy(ot[:, n * 512:(n + 1) * 512], op[:])
            nc.sync.dma_start(O[t * 128:(t + 1) * 128, :], ot[:])

        # ---- prologue: qk projections for token halves 0-1, v chunks 0-7 ----
        for n in (0, 1):
            for (dest, w_r, bbase) in ((qT, wq_r, 0), (kT, wk_r, 2)):
                for m in range(2):
                    qk_unit(dest, w_r, m, n, bbase + m)
        for t in range(8):
            v_unit(t)

        # ---- filler queue consumed inside the attention stream ----
        from collections import deque

        fillers = deque()
        for t in range(8, NT):
            fillers.append(lambda t=t: v_unit(t))
        for n in (2, 3):
            for (dest, w_r, bbase) in ((qT, wq_r, 0), (kT, wk_r, 2)):
                for m in range(2):
                    fillers.append(
                        lambda dest=dest, w_r=w_r, m=m, n=n, b=bbase + m:
                        qk_unit(dest, w_r, m, n, b)
                    )

        def pop_filler():
            if fillers:
                fillers.popleft()()

        # ---- attention blocks ----
        def normalize(hp, J, at):
            # softmax: attT[:, J-slice] = at[0:64] * broadcast(1/at[64]) per head
            for hh in range(2):
                sden = work.tile([1, 512], F32, name="sden", tag="sden", bufs=4)
                nc.vector.tensor_copy(sden[:], at[hh][64:65, :])
                rden = work.tile([1, 512], F32, name="rden", tag="rden", bufs=4)
                nc.vector.reciprocal_approx_fast(rden[:], sden[:])
                rb = work.tile([64, 512], F32, name="rb", tag="rb", bufs=4)
                nc.gpsimd.partition_broadcast(rb[:], rden[:])
                nc.vector.tensor_tensor(
                    attT[hp][hh * 64:(hh + 1) * 64, J * 512:(J + 1) * 512],
                    at[hh][0:64, :],
                    rb[:],
                    op=OP.mult,
                )

        for J, hp in [(0, 0), (0, 1), (1, 0), (1, 1), (2, 0), (2, 1), (3, 0), (3, 1)]:
            at = [
                pav.tile([128, 512], F32, name=f"at{hh}", tag="av", bufs=2)
                for hh in range(2)
            ]

            # chunk order: diagonal kc=4J first (full width, opens the PSUM
            # accumulation), then off-diagonals, then narrow diagonals.
            kcs = [4 * J] + list(range(4 * J)) + [4 * J + i for i in range(1, 4)]

            def issue_sc_exp(kc):
                off = max(0, 128 * (kc - 4 * J))
                w = 512 - off
                sc = psc.tile([128, 1024], F32, name="sc", tag="sc", bufs=2)
                for hh in range(2):
                    nc.tensor.matmul(
                        sc[:, hh * 512:hh * 512 + w],
                        kT[hp][hh * 64:(hh + 1) * 64, kc * 128:(kc + 1) * 128],
                        qT[hp][hh * 64:(hh + 1) * 64, J * 512 + off:(J + 1) * 512],
                        start=True, stop=True,
                        tile_position=(hh * 64, 0),
                    )
                ex = work.tile([128, 1024], BF16, name="ex", tag="ex", bufs=8)
                nc.scalar.activation(
                    ex[:].rearrange("p (h c) -> p h c", c=512)[:, :, 0:w],
                    sc[:].rearrange("p (h c) -> p h c", c=512)[:, :, 0:w],
                    AF.Exp, scale=0.125,
                )
                if off or kc == 4 * J:
                    for hh in range(2):
                        nc.vector.tensor_tensor(
                            ex[:, hh * 512:hh * 512 + 128],
                            ex[:, hh * 512:hh * 512 + 128],
                            tri[:],
                            op=OP.mult,
                        )
                return ex

            def issue_av(kc, ex, first, last):
                off = max(0, 128 * (kc - 4 * J))
                w = 512 - off
                for hh in range(2):
                    h = 2 * hp + hh
                    nc.tensor.matmul(
                        at[hh][0:65, off:512],
                        vt[kc][:, h * 65:(h + 1) * 65],
                        ex[:, hh * 512:hh * 512 + w],
                        start=first, stop=last,
                    )

            prev = None
            for ti, kc in enumerate(kcs):
                ex = issue_sc_exp(kc)
                pop_filler()
                if prev is not None:
                    issue_av(prev[0], prev[1], first=(prev[2] == 0), last=False)
                prev = (kc, ex, ti)
            issue_av(prev[0], prev[1], first=(prev[2] == 0), last=True)

            normalize(hp, J, at)
            if hp == 1:
                for t in range(4 * J, 4 * J + 4):
                    fillers.append(lambda t=t: c_unit(t))

        # ---- tail: drain remaining fillers (O-projection for J=3) ----
        while fillers:
            fillers.popleft()()

    nc.compile()
    _NC_CACHE["nc"] = nc
    return nc


def _prep_core_inputs(H, key_padding_mask, Wq, bq, Wk, bk, Wv, bv, Wo, bo):
    keep = 1.0 - np.asarray(key_padding_mask, dtype=np.float32)  # [B, T]
    bf = ml_dtypes.bfloat16
    in_maps = []
    for c in range(8):
        b, g = divmod(c, 4)
        sl = slice(g * GD, (g + 1) * GD)
        WqT = np.ascontiguousarray(Wq[sl].T)  # [D, GD]
        WkT = np.ascontiguousarray(Wk[sl].T)
        WvT = Wv[sl].T  # [D, GD]
        WvS = np.zeros((D, GV), dtype=np.float32)
        bvS = np.zeros((1, GV), dtype=np.float32)
        for h in range(HPC):
            WvS[:, h * 65:h * 65 + 64] = WvT[:, h * 64:(h + 1) * 64]
            bvS[0, h * 65:h * 65 + 64] = bv[sl][h * 64:(h + 1) * 64]
            bvS[0, h * 65 + 64] = 1.0
        # pack weight k-chunks side by side: [128, NKD*width]
        WqPk = WqT.reshape(NKD, 128, GD).transpose(1, 0, 2).reshape(128, NKD * GD)
        WkPk = WkT.reshape(NKD, 128, GD).transpose(1, 0, 2).reshape(128, NKD * GD)
        WvPk = WvS.reshape(NKD, 128, GV).transpose(1, 0, 2).reshape(128, NKD * GV)
        WoS = np.ascontiguousarray(Wo[:, sl].T)  # [GD, D]
        WoPk = WoS.reshape(2, 128, D).transpose(1, 0, 2).reshape(128, 2 * D)
        bqk_m = np.stack(
            [bq[sl][0:128], bq[sl][128:256], bk[sl][0:128], bk[sl][128:256]], axis=1
        )  # [128, 4]
        in_maps.append({
            "HT": np.ascontiguousarray(H[b].T).astype(bf),
            "WqP": np.ascontiguousarray(WqPk).astype(bf),
            "WkP": np.ascontiguousarray(WkPk).astype(bf),
            "WvP": np.ascontiguousarray(WvPk).astype(bf),
            "WoP": np.ascontiguousarray(WoPk).astype(bf),
            "bqk": np.ascontiguousarray(bqk_m.astype(np.float32)),
            "bvP": bvS.astype(bf),
            "kpm": np.ascontiguousarray(keep[b].reshape(NT, 128).T),
        })
    return in_maps


def kernel(H, key_padding_mask, Wq, bq, Wk, bk, Wv, bv, Wo, bo, _run_kwargs=None):
    H = np.asarray(H, dtype=np.float32)
    Wq = np.asarray(Wq, dtype=np.float32)
    Wk = np.asarray(Wk, dtype=np.float32)
    Wv = np.asarray(Wv, dtype=np.float32)
    Wo = np.asarray(Wo, dtype=np.float32)
    bq = np.asarray(bq, dtype=np.float32)
    bk = np.asarray(bk, dtype=np.float32)
    bv = np.asarray(bv, dtype=np.float32)
    bo = np.asarray(bo, dtype=np.float32)

    nc = build()
    in_maps = _prep_core_inputs(H, key_padding_mask, Wq, bq, Wk, bk, Wv, bv, Wo, bo)
    res = bass_utils.run_bass_kernel_spmd(
        nc, in_maps, core_ids=list(range(8)), **(_run_kwargs or {})
    )
    out = np.zeros((B, T, D), dtype=np.float32)
    for c in range(8):
        out[c // 4] += np.asarray(res.results[c]["O"], dtype=np.float32)
    out += bo
    if _run_kwargs:
        kernel.last_result = res
    return out
